# revision 8
# baseline (speedup 1.0000x reference)
"""Trainium2 Bass kernel for batched multi-head attention.

Problem: B=8, H=8, S=2048, D=64 f32 attention,
  out = softmax(Q K^T / 64**0.25) V  per (b, h).

Sharding: the 64 (b,h) pairs are split 8-per-core across the 8 NeuronCores
(pure data/head parallelism, no collectives).

Per-core algorithm (per head), everything in the k-partitioned orientation so
no large on-chip transposes are needed:
  - Host pre-transposes Q, K to [D, S] (d-major) and casts Q/K/V to bf16.
  - scoresT[k, q] = K^T.T @ Q^T in k-chunks of 128 x q-slabs of 512.
  - exp on the Scalar engine straight out of PSUM with the 1/64**0.25 scale
    folded in (no max subtraction: |scores/tau| <= ~20, safe in f32).
  - AV keeps expT as the *moving* operand (fast path through the PE) with
    V as the stationary, augmented with a ones column so the softmax
    denominators fall out of the same accumulation: PSUM outT[0:64, q]
    unnormalized, outT[64, q] = sum.  Output stays d-major.
  - Softmax denominators cross partitions via a DRAM bounce (store row,
    reload 128-partition-tiled), reciprocal on DVE, broadcast back with a
    stride-0 DMA, one tensor_tensor multiply normalizes.
  - Host transposes the [D, S] outputs back to [S, D] (free).
"""
import sys

sys.path.insert(0, "/opt/trn_rl_repo")

from contextlib import ExitStack

import ml_dtypes
import numpy as np

import concourse.bass as bass
import concourse.tile as tile
from concourse import bacc, mybir
from concourse.bass_utils import run_bass_kernel_spmd

B, H, S, D = 8, 8, 2048, 64
N_CORES = 8
HPC = B * H // N_CORES  # heads per core = 8
SCALE = 1.0 / (D**0.5) ** 0.5  # 1 / 64**0.25
PCHUNK = 128  # k rows per chunk
NCHUNK = S // PCHUNK  # 16
SLAB = 512  # q columns per QK matmul / AV moving tile
NSLAB = S // SLAB  # 4
BF16 = mybir.dt.bfloat16
F32 = mybir.dt.float32

_COMPILED = {}


def build_kernel():
    nc = bacc.Bacc("TRN2", target_bir_lowering=False, debug=False)
    qt = nc.dram_tensor("q_t", [HPC, D, S], BF16, kind="ExternalInput").ap()
    kt = nc.dram_tensor("k_t", [HPC, D, S], BF16, kind="ExternalInput").ap()
    v = nc.dram_tensor("v", [HPC, S, D], BF16, kind="ExternalInput").ap()
    out = nc.dram_tensor("out_t", [HPC, D, S], F32, kind="ExternalOutput").ap()
    # DRAM bounce buffers for the cross-partition softmax-denominator move
    s_dram = nc.dram_tensor("s_scratch", [HPC, S], F32).ap()
    r_dram = nc.dram_tensor("r_scratch", [HPC, S], F32).ap()

    with tile.TileContext(nc) as tc, ExitStack() as ctx:
        qk_pool = ctx.enter_context(tc.tile_pool(name="qk", bufs=2))
        v_pool = ctx.enter_context(tc.tile_pool(name="vp", bufs=2))
        exp_pool = ctx.enter_context(tc.tile_pool(name="exp", bufs=2))
        ot_pool = ctx.enter_context(tc.tile_pool(name="ot", bufs=2))
        small_pool = ctx.enter_context(tc.tile_pool(name="small", bufs=2))
        const_pool = ctx.enter_context(tc.tile_pool(name="const", bufs=1))
        psqk_pool = ctx.enter_context(
            tc.tile_pool(name="psqk", bufs=2, space="PSUM")
        )
        psav_pool = ctx.enter_context(
            tc.tile_pool(name="psav", bufs=2, space="PSUM")
        )

        zbias = const_pool.tile([128, 1], F32)
        nc.vector.memset(zbias[:], 0.0)

        for h in range(HPC):
            # duplicate Q^T/K^T into partitions 64..127 so chunk pairs can be
            # row-packed onto the PE (two K=64 matmuls run concurrently in
            # disjoint 64-row strips of the 128x128 array)
            qt_sb = qk_pool.tile([2 * D, S], BF16, tag="qt")
            nc.sync.dma_start(qt_sb[0:D, :], qt[h])
            nc.sync.dma_start(qt_sb[D : 2 * D, :], qt[h])
            kt_sb = qk_pool.tile([2 * D, S], BF16, tag="kt")
            nc.sync.dma_start(kt_sb[0:D, :], kt[h])
            nc.sync.dma_start(kt_sb[D : 2 * D, :], kt[h])
            v_aug = v_pool.tile([PCHUNK, NCHUNK, D + 1], BF16, tag="vaug")
            nc.scalar.dma_start(
                v_aug[:, :, 0:D], v[h].rearrange("(c p) d -> p c d", p=PCHUNK)
            )
            nc.vector.memset(v_aug[:, :, D : D + 1], 1.0)

            ot_sb = None
            for s in range(NSLAB):
                if s % 2 == 0:
                    # unnormalized outT + sums row for two slabs; the
                    # normalize/store chain runs per slab-pair so it overlaps
                    # the next pair's compute instead of serializing at the end
                    ot_sb = ot_pool.tile([D + 1, 2 * SLAB], F32, tag="ot")
                expT = exp_pool.tile([PCHUNK, NCHUNK, SLAB], BF16, tag="expT")
                for pair in range(NCHUNK // 2):
                    ps = psqk_pool.tile([PCHUNK, 2, SLAB], F32, tag="psqk")
                    for half in range(2):
                        c = 2 * pair + half
                        base = half * D  # chunk a in rows 0-63, chunk b in 64-127
                        nc.tensor.matmul(
                            ps[:, half, :],
                            kt_sb[base : base + D, c * PCHUNK : (c + 1) * PCHUNK],
                            qt_sb[base : base + D, s * SLAB : (s + 1) * SLAB],
                            start=True,
                            stop=True,
                        )
                    nc.scalar.activation(
                        expT[:, 2 * pair : 2 * pair + 2, :],
                        ps[:],
                        mybir.ActivationFunctionType.Exp,
                        bias=zbias[:],
                        scale=SCALE,
                    )
                # AV: V_aug stationary, expT moving -> transposed output
                po = psav_pool.tile([D + 1, SLAB], F32, tag="psav")
                for c in range(NCHUNK):
                    nc.tensor.matmul(
                        po[:],
                        v_aug[:, c, :],
                        expT[:, c, :],
                        start=(c == 0),
                        stop=(c == NCHUNK - 1),
                    )
                nc.vector.tensor_copy(
                    ot_sb[:, (s % 2) * SLAB : (s % 2 + 1) * SLAB], po[:]
                )

                if s % 2 == 0:
                    continue
                # normalize + store for this slab pair (q range [q0, q0+1024))
                p2 = 2 * SLAB
                q0 = (s - 1) * SLAB
                nch = p2 // 128  # 8 chunks of 128 q values
                # scatter the sums row across partitions via a DRAM bounce
                nc.sync.dma_start(
                    s_dram[h][q0 : q0 + p2], ot_sb[D : D + 1, :]
                )
                sums_nat = small_pool.tile([128, nch], F32, tag="sums")
                nc.sync.dma_start(
                    sums_nat[:],
                    s_dram[h][q0 : q0 + p2].rearrange("(c p) -> p c", p=128),
                )
                r_nat = small_pool.tile([128, nch], F32, tag="rnat")
                nc.vector.reciprocal(r_nat[:], sums_nat[:])
                nc.sync.dma_start(
                    bass.AP(r_dram.tensor, h * S + q0, [[1, 128], [128, nch]]),
                    r_nat[:],
                )
                r_bcast = small_pool.tile([D, p2], F32, tag="rbcast")
                nc.sync.dma_start(
                    r_bcast[:],
                    bass.AP(r_dram.tensor, h * S + q0, [[0, D], [1, p2]]),
                )
                o_fin = ot_pool.tile([D, p2], F32, tag="ofin")
                nc.vector.tensor_tensor(
                    o_fin[:], ot_sb[0:D, :], r_bcast[:], op=mybir.AluOpType.mult
                )
                nc.sync.dma_start(out[h][:, q0 : q0 + p2], o_fin[:])
    nc.compile()
    return nc


def _get_compiled():
    if "nc" not in _COMPILED:
        _COMPILED["nc"] = build_kernel()
    return _COMPILED["nc"]


def kernel(query, key, value, _want_results=False):
    nc = _get_compiled()
    q = np.asarray(query).reshape(B * H, S, D)
    k = np.asarray(key).reshape(B * H, S, D)
    v = np.asarray(value).reshape(B * H, S, D)
    in_maps = []
    for c in range(N_CORES):
        sl = slice(c * HPC, (c + 1) * HPC)
        in_maps.append(
            {
                "q_t": np.ascontiguousarray(q[sl].transpose(0, 2, 1)).astype(
                    ml_dtypes.bfloat16
                ),
                "k_t": np.ascontiguousarray(k[sl].transpose(0, 2, 1)).astype(
                    ml_dtypes.bfloat16
                ),
                "v": np.ascontiguousarray(v[sl]).astype(ml_dtypes.bfloat16),
            }
        )
    res = run_bass_kernel_spmd(nc, in_maps, core_ids=list(range(N_CORES)))
    out = np.concatenate(
        [
            res.results[c]["out_t"].transpose(0, 2, 1).reshape(1, HPC, S, D)
            for c in range(N_CORES)
        ],
        axis=0,
    ).reshape(B, H, S, D)
    if _want_results:
        return out, res
    return out


if __name__ == "__main__":
    rng = np.random.default_rng(0)
    q = rng.standard_normal((B, H, S, D), dtype=np.float32)
    k = rng.standard_normal((B, H, S, D), dtype=np.float32)
    v = rng.standard_normal((B, H, S, D), dtype=np.float32)
    o = kernel(q, k, v)
    print("kernel output", o.shape, o.dtype)
